# revision 9
# baseline (speedup 1.0000x reference)
"""Trainium2 Bass kernel for nn_Bottleneck (gnn_message_passing).

Computation (per reference):
  agg = einsum('bjnm,bcm->bjcn', W, x)        # graph aggregation, [B,J*C,N]
  a   = relu(bn1(agg))                         # sync-BN over (B,N)
  h   = conv1x1(a) + b1                        # [B,INTER,N]
  hr  = relu(bn2(h))                           # sync-BN over (B,N)
  out = conv1x1(hr) + b2                       # [B,F,N]
  return (W, concat([x, out], ch))

Distribution: data-parallel over batch B across 8 NeuronCores (one batch
element per core). Params replicated. BN statistics are exact sync-BN:
per-core partial (sum, sumsq) per channel, AllReduce'd across the 8 cores.

Per-core device kernel:
  - W[b] is staged (by the host) transposed per operator j as wT[j] = W[b,j].T
    with shape [m, n]; DMA'd in contiguous row-tiles so the contraction axis m
    lands on SBUF partitions (the tensor engine contracts over partitions).
    Host may also downcast W/x to fp16 (w_dtype) — halves HBM traffic and
    runs the PE at 1 cycle/column instead of fp32's 4.
  - gmul: stationary x~ tiles [m=128, c=32], moving wT tiles [m=128, n=512],
    PSUM-accumulated over the 16 m-tiles into one 4-bank PSUM tile per j.
  - A tiny dummy AllReduce at kernel start absorbs the ncfw cold-start
    (~12us) so the two real BN AllReduces run at their ~10us floor.
  - BN coefficients: s = gamma*rsqrt(var+eps), t = beta - mean*s, applied
    fused in one ACT pass per row-block: relu(s*x + t).
"""

import numpy as np

import concourse.bacc as bacc
import concourse.mybir as mybir
import concourse.tile as tile
from concourse.bass_utils import run_bass_kernel_spmd

F32 = mybir.dt.float32
AF = mybir.ActivationFunctionType
AX = mybir.AxisListType

B, J, N, C, F, INTER = 8, 3, 2048, 32, 32, 128
JC = J * C
BN_EPS = 1e-5
N_CORES = 8

_nc_cache = {}


def _chunk_schedule(mt, first_small, big):
    """m-tile group sizes for one operator's W stream."""
    out = [1, 1, 2, 4] if first_small else []
    left = mt - sum(out)
    while left:
        g = min(big, left)
        out.append(g)
        left -= g
    return out


def build_nc(n=N, w_dtype="float16", big=8):
    key = (n, w_dtype, big)
    if key in _nc_cache:
        return _nc_cache[key]
    WDT = getattr(mybir.dt, w_dtype)

    mt = n // 128            # m-tiles per operator
    nch = n // 512           # 512-wide output chunks
    count = float(B * n)     # BN reduction count (global over cores)

    nc = bacc.Bacc("TRN2", target_bir_lowering=False, debug=False,
                   enable_asserts=False, num_devices=N_CORES)

    wT = nc.dram_tensor("wT", [J, n, n], WDT, kind="ExternalInput").ap()
    xs_d = nc.dram_tensor("xs", [128, mt * C], WDT, kind="ExternalInput").ap()
    w1T_d = nc.dram_tensor("w1T", [JC, INTER], F32, kind="ExternalInput").ap()
    b1_d = nc.dram_tensor("b1", [INTER, 1], F32, kind="ExternalInput").ap()
    w2T_d = nc.dram_tensor("w2T", [INTER, F], F32, kind="ExternalInput").ap()
    b2_d = nc.dram_tensor("b2", [F, 1], F32, kind="ExternalInput").ap()
    g1b1_d = nc.dram_tensor("g1b1", [JC, 2], F32, kind="ExternalInput").ap()
    g2b2_d = nc.dram_tensor("g2b2", [INTER, 2], F32, kind="ExternalInput").ap()
    out_d = nc.dram_tensor("out", [F, n], F32, kind="ExternalOutput").ap()

    with tile.TileContext(nc) as tc:
        with tc.tile_pool(name="consts", bufs=1) as cpool, \
             tc.tile_pool(name="wstage", bufs=1) as wpool, \
             tc.tile_pool(name="work", bufs=1) as work, \
             tc.tile_pool(name="sq", bufs=1) as sqpool, \
             tc.tile_pool(name="small", bufs=1) as small, \
             tc.tile_pool(name="ps", bufs=2, space="PSUM") as pspool, \
             tc.tile_pool(name="dram", bufs=1, space="DRAM") as dram:

            # ---- constants ----
            xs = cpool.tile([128, mt * C], WDT)
            nc.sync.dma_start(xs[:], xs_d[:])
            w1sb = cpool.tile([JC, INTER], F32)
            nc.scalar.dma_start(w1sb[:], w1T_d[:])
            b1sb = cpool.tile([INTER, 1], F32)
            nc.scalar.dma_start(b1sb[:], b1_d[:])
            w2sb = cpool.tile([INTER, F], F32)
            nc.scalar.dma_start(w2sb[:], w2T_d[:])
            b2sb = cpool.tile([F, 1], F32)
            nc.scalar.dma_start(b2sb[:], b2_d[:])
            gb1s = []
            for j in range(J):
                gbj = cpool.tile([C, 2], F32, name=f"gb1c{j}")
                nc.scalar.dma_start(gbj[:], g1b1_d[32 * j:32 * (j + 1), :])
                gb1s.append(gbj)
            g2b2 = cpool.tile([INTER, 2], F32)
            nc.scalar.dma_start(g2b2[:], g2b2_d[:])

            agg = work.tile([JC, n], F32)
            h_sb = work.tile([INTER, n], F32)
            relu_buf = work.tile([INTER, n], F32)
            out_sb = work.tile([F, n], F32)
            st2 = work.tile([INTER, 2], F32)
            s1_all = work.tile([JC, 1], F32)
            t1_all = work.tile([JC, 1], F32)

            def bn_coefs(gst, gb, p, tagp):
                """s = gamma*rsqrt(var+eps); t = beta - mean*s  (per-partition)."""
                mean = small.tile([p, 1], F32, name=f"mean{tagp}")
                nc.vector.tensor_scalar_mul(mean[:], gst[:, 0:1], 1.0 / count)
                ex2 = small.tile([p, 1], F32, name=f"ex2{tagp}")
                nc.vector.tensor_scalar_mul(ex2[:], gst[:, 1:2], 1.0 / count)
                msq = small.tile([p, 1], F32, name=f"msq{tagp}")
                nc.vector.tensor_mul(msq[:], mean[:], mean[:])
                var = small.tile([p, 1], F32, name=f"var{tagp}")
                nc.vector.tensor_sub(var[:], ex2[:], msq[:])
                nc.vector.tensor_scalar_add(var[:], var[:], BN_EPS)
                inv = small.tile([p, 1], F32, name=f"inv{tagp}")
                nc.vector.reciprocal(inv[:], var[:])
                rs = small.tile([p, 1], F32, name=f"rs{tagp}")
                nc.scalar.activation(rs[:], inv[:], AF.Sqrt)
                s = small.tile([p, 1], F32, name=f"s{tagp}")
                nc.vector.tensor_mul(s[:], rs[:], gb[:, 0:1])
                t = small.tile([p, 1], F32, name=f"t{tagp}")
                nc.vector.tensor_mul(t[:], mean[:], s[:])
                nc.vector.tensor_sub(t[:], gb[:, 1:2], t[:])
                return s, t

            # ---- phase A: gmul; per-j evac + stats + AllReduce trigger ----
            ar1_outs = []
            big_i = [0]
            for j in range(J):
                rows = slice(32 * j, 32 * (j + 1))
                psj = pspool.tile([C, n], F32, name="psj", tag="ps")
                k = 0
                for g in _chunk_schedule(mt, j == 0, big):
                    if g == big:
                        ab = "A" if big_i[0] % 2 == 0 else "B"
                        big_i[0] += 1
                        wt = wpool.tile([128, g, n], WDT, name=f"wt8{ab}",
                                        tag=f"wt8{ab}", bufs=(2 if ab == "A" else 1))
                        eng = nc.sync if ab == "A" else nc.scalar
                    else:
                        wt = wpool.tile([128, g, n], WDT, name=f"wt{g}",
                                        tag=f"wt{g}",
                                        bufs=(2 if g == 1 else 1))
                        eng = nc.sync
                    src = wT[j, k * 128:(k + g) * 128, :]
                    eng.dma_start(
                        wt[:], src.rearrange("(t p) n -> p t n", p=128))
                    for t in range(g):
                        lhs = xs[:, (k + t) * C:(k + t + 1) * C]
                        for c in range(nch):
                            nc.tensor.matmul(
                                psj[:, c * 512:(c + 1) * 512], lhs,
                                wt[:, t, c * 512:(c + 1) * 512],
                                start=(k + t == 0), stop=(k + t == mt - 1))
                    k += g
                sq = sqpool.tile([128, n], F32, name="sq", tag="sq")
                if j < 2:
                    nc.vector.tensor_copy(agg[rows, :], psj[:])
                    nc.vector.tensor_mul(sq[rows, :], agg[rows, :], agg[rows, :])
                else:
                    nc.scalar.copy(agg[rows, :], psj[:])
                    nc.scalar.activation(sq[rows, :], agg[rows, :], AF.Square)
                st1j = small.tile([C, 2], F32, name=f"st1{j}")
                nc.vector.reduce_sum(st1j[:, 0:1], agg[rows, :], axis=AX.X)
                nc.vector.reduce_sum(st1j[:, 1:2], sq[rows, :], axis=AX.X)
                arj_in = dram.tile([C, 2], F32, name=f"ar1i{j}")
                arj_out = dram.tile([C, 2], F32, name=f"ar1o{j}",
                                    addr_space="Shared")
                nc.gpsimd.dma_start(arj_in[:], st1j[:])
                nc.gpsimd.collective_compute(
                    "AllReduce", mybir.AluOpType.add,
                    replica_groups=[list(range(N_CORES))],
                    ins=[arj_in.opt()], outs=[arj_out.opt()])
                ar1_outs.append(arj_out)

            # ---- post-AR bn1: gather stats, coefs, relu-apply (per j) ----
            for j in range(J):
                rows = slice(32 * j, 32 * (j + 1))
                gstj = small.tile([C, 2], F32, name=f"gst1{j}")
                nc.scalar.dma_start(gstj[:], ar1_outs[j][:])
                sj, tj = bn_coefs(gstj, gb1s[j], C, f"1{j}")
                nc.scalar.copy(s1_all[rows, :], sj[:])
                nc.scalar.copy(t1_all[rows, :], tj[:])
                for c in range(nch):
                    cols = slice(c * 512, (c + 1) * 512)
                    nc.scalar.activation(relu_buf[rows, cols], agg[rows, cols],
                                         AF.Relu, bias=t1_all[rows, :],
                                         scale=s1_all[rows, :])

            # ---- conv1 + b1 in halves; bn2 local stats per half ----
            st2p = small.tile([INTER, 4], F32)
            for hh in range(2):
                hcols = slice(hh * (n // 2), (hh + 1) * (n // 2))
                hp = pspool.tile([INTER, n // 2], F32, name="hp", tag="ps")
                for c in range(nch // 2):
                    lo = hh * (n // 2) + c * 512
                    nc.tensor.matmul(hp[:, c * 512:(c + 1) * 512], w1sb[:],
                                     relu_buf[:JC, lo:lo + 512],
                                     start=True, stop=True)
                nc.scalar.activation(h_sb[:, hcols], hp[:], AF.Identity,
                                     bias=b1sb[:])
                nc.vector.reduce_sum(st2p[:, hh:hh + 1], h_sb[:, hcols],
                                     axis=AX.X)
                sq = sqpool.tile([128, n], F32, name="sq", tag="sq")
                nc.scalar.activation(sq[:, hcols], h_sb[:, hcols], AF.Square)
                nc.vector.reduce_sum(st2p[:, 2 + hh:3 + hh], sq[:, hcols],
                                     axis=AX.X)
            nc.vector.reduce_sum(st2[:, 0:1], st2p[:, 0:2], axis=AX.X)
            nc.vector.reduce_sum(st2[:, 1:2], st2p[:, 2:4], axis=AX.X)

            # ---- bn2 stats all-reduce ----
            ar2_in = dram.tile([INTER, 2], F32)
            ar2_out = dram.tile([INTER, 2], F32, addr_space="Shared")
            nc.gpsimd.dma_start(ar2_in[:], st2[:])
            nc.gpsimd.collective_compute(
                "AllReduce", mybir.AluOpType.add,
                replica_groups=[list(range(N_CORES))],
                ins=[ar2_in.opt()], outs=[ar2_out.opt()])
            gst2 = small.tile([INTER, 2], F32)
            nc.scalar.dma_start(gst2[:], ar2_out[:])

            s2, t2 = bn_coefs(gst2, g2b2, INTER, "2")

            # ---- hr = relu(s2*h + t2); conv2 + b2; out, in halves ----
            for hh in range(2):
                hcols = slice(hh * (n // 2), (hh + 1) * (n // 2))
                for c in range(nch // 2):
                    lo = hh * (n // 2) + c * 512
                    nc.scalar.activation(relu_buf[:, lo:lo + 512],
                                         h_sb[:, lo:lo + 512], AF.Relu,
                                         bias=t2[:], scale=s2[:])
                op = pspool.tile([F, n // 2], F32, name="op", tag="ps")
                for c in range(nch // 2):
                    lo = hh * (n // 2) + c * 512
                    nc.tensor.matmul(op[:, c * 512:(c + 1) * 512], w2sb[:],
                                     relu_buf[:, lo:lo + 512],
                                     start=True, stop=True)
                nc.scalar.activation(out_sb[:, hcols], op[:], AF.Identity,
                                     bias=b2sb[:])
                nc.sync.dma_start(out_d[:, hcols], out_sb[:, hcols])

    nc.compile()
    _nc_cache[key] = nc
    return nc


def make_in_maps(W, x, bn1_gamma, bn1_beta, conv1_w, conv1_b,
                 bn2_gamma, bn2_beta, conv2_w, conv2_b, n=N,
                 w_dtype="float16"):
    mt = n // 128
    wnp = np.dtype(w_dtype)
    w1T = np.ascontiguousarray(conv1_w.T)
    w2T = np.ascontiguousarray(conv2_w.T)
    b1 = np.ascontiguousarray(conv1_b[:, None])
    b2 = np.ascontiguousarray(conv2_b[:, None])
    g1b1 = np.ascontiguousarray(np.stack([bn1_gamma, bn1_beta], axis=1))
    g2b2 = np.ascontiguousarray(np.stack([bn2_gamma, bn2_beta], axis=1))
    in_maps = []
    for b in range(B):
        wTb = np.ascontiguousarray(W[b].transpose(0, 2, 1).astype(wnp))
        xsb = np.ascontiguousarray(
            x[b, :, :, 0].T.reshape(mt, 128, C).transpose(1, 0, 2)
        ).reshape(128, mt * C).astype(wnp)
        in_maps.append({
            "wT": wTb, "xs": xsb, "w1T": w1T, "b1": b1, "w2T": w2T, "b2": b2,
            "g1b1": g1b1, "g2b2": g2b2,
        })
    return in_maps


def run(inputs, n=N, w_dtype="float16", big=8, trace=False, trace_cores=None):
    """Returns ((W, out_full), BassKernelResults)."""
    inputs = {k: np.asarray(v, dtype=np.float32) for k, v in inputs.items()}
    nc = build_nc(n=n, w_dtype=w_dtype, big=big)
    in_maps = make_in_maps(n=n, w_dtype=w_dtype, **inputs)
    res = run_bass_kernel_spmd(nc, in_maps, core_ids=list(range(N_CORES)),
                               trace=trace, trace_cores=trace_cores)
    x = inputs["x"]
    out_full = np.empty((B, C + F, n, 1), dtype=np.float32)
    out_full[:, :C] = x
    for b in range(B):
        out_full[b, C:] = res.results[b]["out"][:, :, None]
    return (inputs["W"], out_full), res


def kernel(**inputs):
    out, _ = run(inputs)
    return out


# revision 10
# speedup vs baseline: 1.2238x; 1.2238x over previous
"""Trainium2 Bass kernel for nn_Bottleneck (gnn_message_passing).

Computation (per reference):
  agg = einsum('bjnm,bcm->bjcn', W, x)        # graph aggregation, [B,J*C,N]
  a   = relu(bn1(agg))                         # sync-BN over (B,N)
  h   = conv1x1(a) + b1                        # [B,INTER,N]
  hr  = relu(bn2(h))                           # sync-BN over (B,N)
  out = conv1x1(hr) + b2                       # [B,F,N]
  return (W, concat([x, out], ch))

Distribution: data-parallel over batch B across 8 NeuronCores (one batch
element per core). Params replicated. BN statistics are exact sync-BN:
per-core partial (sum, sumsq) per channel, AllReduce'd across the 8 cores.

Per-core device kernel:
  - W[b] is staged (by the host) transposed per operator j as wT[j] = W[b,j].T
    with shape [m, n]; DMA'd in contiguous row-tiles so the contraction axis m
    lands on SBUF partitions (the tensor engine contracts over partitions).
    Host may also downcast W/x to fp16 (w_dtype) — halves HBM traffic and
    runs the PE at 1 cycle/column instead of fp32's 4.
  - gmul: stationary x~ tiles [m=128, c=32], moving wT tiles [m=128, n=512],
    PSUM-accumulated over the 16 m-tiles into one 4-bank PSUM tile per j.
  - A tiny dummy AllReduce at kernel start absorbs the ncfw cold-start
    (~12us) so the two real BN AllReduces run at their ~10us floor.
  - BN coefficients: s = gamma*rsqrt(var+eps), t = beta - mean*s, applied
    fused in one ACT pass per row-block: relu(s*x + t).
"""

import numpy as np

import concourse.bacc as bacc
import concourse.mybir as mybir
import concourse.tile as tile
from concourse.bass_utils import run_bass_kernel_spmd

F32 = mybir.dt.float32
AF = mybir.ActivationFunctionType
AX = mybir.AxisListType

B, J, N, C, F, INTER = 8, 3, 2048, 32, 32, 128
JC = J * C
BN_EPS = 1e-5
N_CORES = 8

_nc_cache = {}


def _chunk_schedule(mt, first_small, big):
    """m-tile group sizes for one operator's W stream."""
    out = [1, 1, 2, 4] if first_small else []
    left = mt - sum(out)
    while left:
        g = min(big, left)
        out.append(g)
        left -= g
    return out


def build_nc(n=N, w_dtype="float16", big=4):
    key = (n, w_dtype, big)
    if key in _nc_cache:
        return _nc_cache[key]
    WDT = getattr(mybir.dt, w_dtype)

    mt = n // 128            # m-tiles per operator
    nch = n // 512           # 512-wide output chunks
    count = float(B * n)     # BN reduction count (global over cores)

    nc = bacc.Bacc("TRN2", target_bir_lowering=False, debug=False,
                   enable_asserts=False, num_devices=N_CORES)

    wT = nc.dram_tensor("wT", [J, n, n], WDT, kind="ExternalInput").ap()
    xs_d = nc.dram_tensor("xs", [128, mt * C], WDT, kind="ExternalInput").ap()
    w1T_d = nc.dram_tensor("w1T", [JC, INTER], F32, kind="ExternalInput").ap()
    b1_d = nc.dram_tensor("b1", [INTER, 1], F32, kind="ExternalInput").ap()
    w2T_d = nc.dram_tensor("w2T", [INTER, F], F32, kind="ExternalInput").ap()
    b2_d = nc.dram_tensor("b2", [F, 1], F32, kind="ExternalInput").ap()
    g1b1_d = nc.dram_tensor("g1b1", [JC, 2], F32, kind="ExternalInput").ap()
    g2b2_d = nc.dram_tensor("g2b2", [INTER, 2], F32, kind="ExternalInput").ap()
    out_d = nc.dram_tensor("out", [F, n], F32, kind="ExternalOutput").ap()

    with tile.TileContext(nc) as tc:
        with tc.tile_pool(name="consts", bufs=1) as cpool, \
             tc.tile_pool(name="wstage", bufs=1) as wpool, \
             tc.tile_pool(name="work", bufs=1) as work, \
             tc.tile_pool(name="sq", bufs=1) as sqpool, \
             tc.tile_pool(name="small", bufs=1) as small, \
             tc.tile_pool(name="ps", bufs=2, space="PSUM") as pspool, \
             tc.tile_pool(name="dram", bufs=1, space="DRAM") as dram:

            # ---- constants ----
            xs = cpool.tile([128, mt * C], WDT)
            nc.sync.dma_start(xs[:], xs_d[:])
            w1sb = cpool.tile([JC, INTER], F32)
            nc.scalar.dma_start(w1sb[:], w1T_d[:])
            b1sb = cpool.tile([INTER, 1], F32)
            nc.scalar.dma_start(b1sb[:], b1_d[:])
            w2sb = cpool.tile([INTER, F], F32)
            nc.scalar.dma_start(w2sb[:], w2T_d[:])
            b2sb = cpool.tile([F, 1], F32)
            nc.scalar.dma_start(b2sb[:], b2_d[:])
            gb1s = []
            for j in range(J):
                gbj = cpool.tile([C, 2], F32, name=f"gb1c{j}")
                nc.scalar.dma_start(gbj[:], g1b1_d[32 * j:32 * (j + 1), :])
                gb1s.append(gbj)
            g2b2 = cpool.tile([INTER, 2], F32)
            nc.scalar.dma_start(g2b2[:], g2b2_d[:])

            agg = work.tile([JC, n], F32)
            h_sb = work.tile([INTER, n], F32)
            relu_buf = work.tile([INTER, n], F32)
            out_sb = work.tile([F, n], F32)
            st2 = work.tile([INTER, 2], F32)
            s1_all = work.tile([JC, 1], F32)
            t1_all = work.tile([JC, 1], F32)

            def bn_coefs(gst, gb, p, tagp):
                """s = gamma*rsqrt(var+eps); t = beta - mean*s  (per-partition)."""
                mean = small.tile([p, 1], F32, name=f"mean{tagp}")
                nc.vector.tensor_scalar_mul(mean[:], gst[:, 0:1], 1.0 / count)
                ex2 = small.tile([p, 1], F32, name=f"ex2{tagp}")
                nc.vector.tensor_scalar_mul(ex2[:], gst[:, 1:2], 1.0 / count)
                msq = small.tile([p, 1], F32, name=f"msq{tagp}")
                nc.vector.tensor_mul(msq[:], mean[:], mean[:])
                var = small.tile([p, 1], F32, name=f"var{tagp}")
                nc.vector.tensor_sub(var[:], ex2[:], msq[:])
                nc.vector.tensor_scalar_add(var[:], var[:], BN_EPS)
                inv = small.tile([p, 1], F32, name=f"inv{tagp}")
                nc.vector.reciprocal(inv[:], var[:])
                rs = small.tile([p, 1], F32, name=f"rs{tagp}")
                nc.scalar.activation(rs[:], inv[:], AF.Sqrt)
                s = small.tile([p, 1], F32, name=f"s{tagp}")
                nc.vector.tensor_mul(s[:], rs[:], gb[:, 0:1])
                t = small.tile([p, 1], F32, name=f"t{tagp}")
                nc.vector.tensor_mul(t[:], mean[:], s[:])
                nc.vector.tensor_sub(t[:], gb[:, 1:2], t[:])
                return s, t

            # ---- phase A: gmul; per-j evac + stats + AllReduce trigger ----
            ar1_outs = []
            big_i = [0]
            for j in range(J):
                rows = slice(32 * j, 32 * (j + 1))
                psj = pspool.tile([C, n], F32, name="psj", tag="ps")
                k = 0
                for g in _chunk_schedule(mt, j == 0, big):
                    if g == big:
                        ab = "A" if big_i[0] % 2 == 0 else "B"
                        big_i[0] += 1
                        wt = wpool.tile([128, g, n], WDT, name=f"wt8{ab}",
                                        tag=f"wt8{ab}", bufs=(3 if ab == "A" else 2))
                        eng = nc.sync if ab == "A" else nc.scalar
                    else:
                        wt = wpool.tile([128, g, n], WDT, name=f"wt{g}",
                                        tag=f"wt{g}",
                                        bufs=(2 if g == 1 else 1))
                        eng = nc.sync
                    src = wT[j, k * 128:(k + g) * 128, :]
                    eng.dma_start(
                        wt[:], src.rearrange("(t p) n -> p t n", p=128))
                    for t in range(g):
                        lhs = xs[:, (k + t) * C:(k + t + 1) * C]
                        for c in range(nch):
                            nc.tensor.matmul(
                                psj[:, c * 512:(c + 1) * 512], lhs,
                                wt[:, t, c * 512:(c + 1) * 512],
                                start=(k + t == 0), stop=(k + t == mt - 1))
                    k += g
                sq = sqpool.tile([128, n], F32, name="sq", tag="sq")
                if j < 2:
                    nc.vector.tensor_copy(agg[rows, :], psj[:])
                    nc.vector.tensor_mul(sq[rows, :], agg[rows, :], agg[rows, :])
                else:
                    nc.scalar.copy(agg[rows, :], psj[:])
                    nc.scalar.activation(sq[rows, :], agg[rows, :], AF.Square)
                st1j = small.tile([C, 2], F32, name=f"st1{j}")
                nc.vector.reduce_sum(st1j[:, 0:1], agg[rows, :], axis=AX.X)
                nc.vector.reduce_sum(st1j[:, 1:2], sq[rows, :], axis=AX.X)
                arj_in = dram.tile([C, 2], F32, name=f"ar1i{j}")
                arj_out = dram.tile([C, 2], F32, name=f"ar1o{j}",
                                    addr_space="Shared")
                nc.scalar.dma_start(arj_in[:], st1j[:])
                nc.gpsimd.collective_compute(
                    "AllReduce", mybir.AluOpType.add,
                    replica_groups=[list(range(N_CORES))],
                    ins=[arj_in.opt()], outs=[arj_out.opt()])
                ar1_outs.append(arj_out)

            # ---- post-AR bn1: gather stats, coefs, relu-apply (per j) ----
            for j in range(J):
                rows = slice(32 * j, 32 * (j + 1))
                gstj = small.tile([C, 2], F32, name=f"gst1{j}")
                nc.scalar.dma_start(gstj[:], ar1_outs[j][:])
                sj, tj = bn_coefs(gstj, gb1s[j], C, f"1{j}")
                nc.scalar.copy(s1_all[rows, :], sj[:])
                nc.scalar.copy(t1_all[rows, :], tj[:])
                for c in range(nch):
                    cols = slice(c * 512, (c + 1) * 512)
                    nc.scalar.activation(relu_buf[rows, cols], agg[rows, cols],
                                         AF.Relu, bias=t1_all[rows, :],
                                         scale=s1_all[rows, :])

            # ---- conv1 + b1 in halves; bn2 local stats per half ----
            st2p = small.tile([INTER, 4], F32)
            for hh in range(2):
                hcols = slice(hh * (n // 2), (hh + 1) * (n // 2))
                hp = pspool.tile([INTER, n // 2], F32, name="hp", tag="ps")
                for c in range(nch // 2):
                    lo = hh * (n // 2) + c * 512
                    nc.tensor.matmul(hp[:, c * 512:(c + 1) * 512], w1sb[:],
                                     relu_buf[:JC, lo:lo + 512],
                                     start=True, stop=True)
                nc.scalar.activation(h_sb[:, hcols], hp[:], AF.Identity,
                                     bias=b1sb[:])
                nc.vector.reduce_sum(st2p[:, hh:hh + 1], h_sb[:, hcols],
                                     axis=AX.X)
                sq = sqpool.tile([128, n], F32, name="sq", tag="sq")
                nc.scalar.activation(sq[:, hcols], h_sb[:, hcols], AF.Square)
                nc.vector.reduce_sum(st2p[:, 2 + hh:3 + hh], sq[:, hcols],
                                     axis=AX.X)
            nc.vector.reduce_sum(st2[:, 0:1], st2p[:, 0:2], axis=AX.X)
            nc.vector.reduce_sum(st2[:, 1:2], st2p[:, 2:4], axis=AX.X)

            # ---- bn2 stats all-reduce ----
            ar2_in = dram.tile([INTER, 2], F32)
            ar2_out = dram.tile([INTER, 2], F32, addr_space="Shared")
            nc.scalar.dma_start(ar2_in[:], st2[:])
            nc.gpsimd.collective_compute(
                "AllReduce", mybir.AluOpType.add,
                replica_groups=[list(range(N_CORES))],
                ins=[ar2_in.opt()], outs=[ar2_out.opt()])
            gst2 = small.tile([INTER, 2], F32)
            nc.scalar.dma_start(gst2[:], ar2_out[:])

            s2, t2 = bn_coefs(gst2, g2b2, INTER, "2")

            # ---- hr = relu(s2*h + t2); conv2 + b2; out, in halves ----
            for hh in range(2):
                hcols = slice(hh * (n // 2), (hh + 1) * (n // 2))
                for c in range(nch // 2):
                    lo = hh * (n // 2) + c * 512
                    nc.scalar.activation(relu_buf[:, lo:lo + 512],
                                         h_sb[:, lo:lo + 512], AF.Relu,
                                         bias=t2[:], scale=s2[:])
                op = pspool.tile([F, n // 2], F32, name="op", tag="ps")
                for c in range(nch // 2):
                    lo = hh * (n // 2) + c * 512
                    nc.tensor.matmul(op[:, c * 512:(c + 1) * 512], w2sb[:],
                                     relu_buf[:, lo:lo + 512],
                                     start=True, stop=True)
                nc.scalar.activation(out_sb[:, hcols], op[:], AF.Identity,
                                     bias=b2sb[:])
                nc.sync.dma_start(out_d[:, hcols], out_sb[:, hcols])

    nc.compile()
    _nc_cache[key] = nc
    return nc


def make_in_maps(W, x, bn1_gamma, bn1_beta, conv1_w, conv1_b,
                 bn2_gamma, bn2_beta, conv2_w, conv2_b, n=N,
                 w_dtype="float16"):
    mt = n // 128
    wnp = np.dtype(w_dtype)
    w1T = np.ascontiguousarray(conv1_w.T)
    w2T = np.ascontiguousarray(conv2_w.T)
    b1 = np.ascontiguousarray(conv1_b[:, None])
    b2 = np.ascontiguousarray(conv2_b[:, None])
    g1b1 = np.ascontiguousarray(np.stack([bn1_gamma, bn1_beta], axis=1))
    g2b2 = np.ascontiguousarray(np.stack([bn2_gamma, bn2_beta], axis=1))
    in_maps = []
    for b in range(B):
        wTb = np.ascontiguousarray(W[b].transpose(0, 2, 1).astype(wnp))
        xsb = np.ascontiguousarray(
            x[b, :, :, 0].T.reshape(mt, 128, C).transpose(1, 0, 2)
        ).reshape(128, mt * C).astype(wnp)
        in_maps.append({
            "wT": wTb, "xs": xsb, "w1T": w1T, "b1": b1, "w2T": w2T, "b2": b2,
            "g1b1": g1b1, "g2b2": g2b2,
        })
    return in_maps


def run(inputs, n=N, w_dtype="float16", big=4, trace=False, trace_cores=None):
    """Returns ((W, out_full), BassKernelResults)."""
    inputs = {k: np.asarray(v, dtype=np.float32) for k, v in inputs.items()}
    nc = build_nc(n=n, w_dtype=w_dtype, big=big)
    in_maps = make_in_maps(n=n, w_dtype=w_dtype, **inputs)
    res = run_bass_kernel_spmd(nc, in_maps, core_ids=list(range(N_CORES)),
                               trace=trace, trace_cores=trace_cores)
    x = inputs["x"]
    out_full = np.empty((B, C + F, n, 1), dtype=np.float32)
    out_full[:, :C] = x
    for b in range(B):
        out_full[b, C:] = res.results[b]["out"][:, :, None]
    return (inputs["W"], out_full), res


def kernel(**inputs):
    out, _ = run(inputs)
    return out
